# revision 18
# baseline (speedup 1.0000x reference)
"""Trainium2 Bass kernel for single-head attention.

Reference computation (per batch b):
    q = x @ Wq; k = x @ Wk; v = x @ Wv          # x: [S, D], W: [D, D]
    out = softmax(q @ k.T / sqrt(D)) @ v

Shapes: B=4, S=2048, D=1024, f32.

Sharding over 8 NeuronCores: core c -> (batch b = c//2, seq half h = c%2).
Each core:
  - computes q^T, k^T (layout [e, s]) and v ([s, e]) for its own S/2 rows
  - AllGathers k^T (float32r, 2 chunks) and v (bf16, 1 chunk) within the
    pair {2b, 2b+1}
  - computes scores for its 1024 queries vs all 2048 keys, softmax,
    attn @ v, writes its [1024, 1024] output shard.

dtype strategy (validated empirically):
  - all matmuls in float32r (~13-bit mantissa; end-to-end rel err ~9e-3
    vs the f32 reference, under the 2e-2 gate)
  - attn weights / gathered v in bf16 (error enters output linearly).

v3 scheduling. The first ~150us are DMA-bandwidth-bound (~50MB/core in
v2: 16MB inputs + 6MB staging + collective wire traffic + 12MB
reloading BOTH ranks' gathered data), which also starves the
collectives' own SDMA descriptors. Changes:
  - own k^T/v stay resident in SBUF after projection; only the PEER
    half is loaded from the gather output, via a dynamic-offset DMA
    (row index = 1 - partition_id%2). Attention uses a per-core
    [own rows, peer rows] key order -- softmax over k is
    permutation-invariant, so the output is unchanged.
  - no barrier collective (the CC engine serializes collectives; the
    barrier only delayed kt chunk 0 by ~10us).
  - input loads ride the sync queue in priority order (x/wk
    interleaved, then wv, then wq); staging + output stores ride
    scalar; collective triggers ride gpsimd.
  - attention phase: software-pipelined, PE stream
    scores(i) | transposes(i-1) | attn@v(i-1).
"""

import numpy as np

import concourse.bass as bass
import concourse.mybir as mybir
import concourse.tile as tile
from concourse import bacc
from concourse.bass_utils import run_bass_kernel_spmd

P = 128          # partitions
D = 1024         # model dim (= E)
S_OWN = 1024     # sequence rows per core
S_FULL = 2048
B, NCORES = 4, 8
DT = D // P      # 8 d-tiles
ST = S_OWN // P  # 8 s-tiles
NT = S_FULL // P  # 16 key tiles
F32 = mybir.dt.float32
F32R = mybir.dt.float32r
BF16 = mybir.dt.bfloat16
REPLICA_GROUPS = [[0, 1], [2, 3], [4, 5], [6, 7]]
KT_CHUNKS = 2    # k^T AllGather split (2MB each)
V_CHUNKS = 2     # v AllGather split (1MB each)


def build_kernel():
    nc = bacc.Bacc("TRN2", target_bir_lowering=False, num_devices=NCORES)

    x_d = nc.dram_tensor("x", [S_OWN, D], F32, kind="ExternalInput")
    wq_d = nc.dram_tensor("Wq", [D, D], F32, kind="ExternalInput")
    wk_d = nc.dram_tensor("Wk", [D, D], F32, kind="ExternalInput")
    wv_d = nc.dram_tensor("Wv", [D, D], F32, kind="ExternalInput")
    out_d = nc.dram_tensor("out", [S_OWN, D], F32, kind="ExternalOutput")

    # collective bounce buffers (internal DRAM)
    ec = D // KT_CHUNKS   # e-rows per kT chunk
    kt_send = [nc.dram_tensor(f"kt_send{i}", [ec, S_OWN], F32R)
               for i in range(KT_CHUNKS)]
    kt_allc = [nc.dram_tensor(f"kt_all{i}", [2, ec, S_OWN], F32R)
               for i in range(KT_CHUNKS)]
    sc = S_OWN // V_CHUNKS  # s-rows per v chunk
    v_send = [nc.dram_tensor(f"v_send{i}", [sc, D], BF16)
              for i in range(V_CHUNKS)]
    v_allc = [nc.dram_tensor(f"v_all{i}", [2, sc, D], BF16)
              for i in range(V_CHUNKS)]

    bar_send = nc.dram_tensor("bar_send", [1, 128], F32)
    bar_out = nc.dram_tensor("bar_out", [2, 128], F32)

    ident_np = np.eye(P, dtype=np.float32)
    ident_d = nc.inline_tensor(ident_np, name="ident")

    with tile.TileContext(nc) as tc:
        _emit(nc, tc, x_d, wq_d, wk_d, wv_d, out_d,
              kt_send, kt_allc, v_send, v_allc, ident_d, bar_send, bar_out)
    nc.compile()
    return nc


def _emit(nc, tc, x_d, wq_d, wk_d, wv_d, out_d,
          kt_send, kt_allc, v_send, v_allc, ident_d, bar_send, bar_out):
    with tc.tile_pool(name="sb", bufs=1) as sb:
        ident = sb.tile([P, P], F32, name="ident")
        nc.sync.dma_start(ident[:], ident_d.ap())
        identb = sb.tile([P, P], BF16, name="identb")
        nc.gpsimd.dma_start(identb[:], ident_d.ap())  # cast f32->bf16

        # tiny AllGather at t=0: pays the CC engine's ~35-40us
        # first-collective arming latency during the load phase, so the
        # kT gather processes immediately when its data is staged
        nc.gpsimd.dma_start(bar_send.ap(), ident_d.ap()[0:1, :])
        nc.gpsimd.collective_compute(
            "AllGather", mybir.AluOpType.bypass,
            replica_groups=REPLICA_GROUPS,
            ins=[bar_send.ap().opt()],
            outs=[bar_out.ap().opt()],
        )

        # which gather-output row is the peer's (0 or 1)
        peer = 1 - (nc.sync.partition_id() % 2)

        # SBUF tag plan (KB/partition, 207.9 usable). Generational reuse:
        #   wk0: wk(8x4K)  -> qT(8x4K)        [wk dies at kT-proj end]
        #   wv0: wv(8x4K)  -> kT_peer(8x4K)   [wv dies at v-proj end]
        #   wq0: wq(8x4K)  -> v_peer(8x2K)    [wq dies at q-proj end]
        #   xT0: xT(8x4K)                     [dies at q-proj end]
        #   kTo: own k^T, 8x4K dedicated
        #   vo:  own v, 8x2K dedicated
        #   xa:  x_nat(3 bufs) -> attn(3 bufs); attnT 2 bufs; stage 2 bufs
        wk_sb = [sb.tile([P, D], F32R, name=f"wk{d}", tag="wk0", bufs=8)
                 for d in range(DT)]
        wv_sb = [sb.tile([P, D], F32R, name=f"wv{d}", tag="wv0", bufs=8)
                 for d in range(DT)]
        wq_sb = [sb.tile([P, D], F32R, name=f"wq{d}", tag="wq0", bufs=8)
                 for d in range(DT)]
        xT = [sb.tile([P, S_OWN], F32R, name=f"xT{d}", tag="xT0", bufs=8)
              for d in range(DT)]
        kT_own = [sb.tile([P, S_OWN], F32R, name=f"kTo{e}", tag="kTo",
                          bufs=8) for e in range(DT)]
        v_own = [sb.tile([P, D], BF16, name=f"vo{s}", tag="vo", bufs=8)
                 for s in range(ST)]

        with tc.tile_pool(name="ps1", bufs=1, space="PSUM") as ps1:
            # ---- input loads: one queue, priority order x/wk, wv, wq ----
            x_nats = []
            for s in range(ST):
                x_nat = sb.tile([P, D], F32, name=f"x_nat{s}", tag="xa",
                                bufs=3)
                eng = nc.sync if s % 2 == 0 else nc.scalar
                eng.dma_start(x_nat[:], x_d.ap()[s * P:(s + 1) * P, :])
                x_nats.append(x_nat)
            for d in range(DT):
                nc.sync.dma_start(
                    wk_sb[d][:], wk_d.ap()[d * P:(d + 1) * P, :].bitcast(F32R))
            for d in range(DT):
                nc.sync.dma_start(
                    wv_sb[d][:], wv_d.ap()[d * P:(d + 1) * P, :].bitcast(F32R))
            for d in range(DT):
                nc.sync.dma_start(
                    wq_sb[d][:], wq_d.ap()[d * P:(d + 1) * P, :].bitcast(F32R))

            # ---- x transposes (PE) as tiles arrive ----
            for s in range(ST):
                x_nat = x_nats[s]
                for d in range(DT):
                    pt = ps1.tile([P, P], F32, name=f"pt{s}_{d}", tag="pt",
                                  bufs=2)
                    nc.tensor.transpose(pt[:], x_nat[:, d * P:(d + 1) * P],
                                        ident[:])
                    nc.vector.tensor_copy(xT[d][:, s * P:(s + 1) * P], pt[:])

            # ---- k^T projection -> own SBUF -> DRAM -> AllGather ASAP ----
            epc = DT // KT_CHUNKS
            for ch in range(KT_CHUNKS):
                for ei in range(epc):
                    e = ch * epc + ei
                    pk = ps1.tile([P, S_OWN], F32, name=f"pk{e}", tag="proj",
                                  bufs=3)
                    for d in range(DT):
                        for c in range(2):
                            nc.tensor.matmul(
                                pk[:, c * 512:(c + 1) * 512],
                                wk_sb[d][:, e * P:(e + 1) * P],
                                xT[d][:, c * 512:(c + 1) * 512],
                                start=(d == 0), stop=(d == DT - 1))
                    nc.vector.tensor_copy(kT_own[e][:], pk[:])
                    nc.scalar.dma_start(
                        kt_send[ch].ap()[ei * P:(ei + 1) * P, :], kT_own[e][:])
                nc.gpsimd.collective_compute(
                    "AllGather", mybir.AluOpType.bypass,
                    replica_groups=REPLICA_GROUPS,
                    ins=[kt_send[ch].ap().opt()],
                    outs=[kt_allc[ch].ap().opt()],
                )

            # ---- v projection -> own SBUF (bf16) -> DRAM -> AllGather ----
            spc = ST // V_CHUNKS
            for ch in range(V_CHUNKS):
                for si in range(spc):
                    s = ch * spc + si
                    pv = ps1.tile([P, D], F32, name=f"pv{s}", tag="proj",
                                  bufs=3)
                    for d in range(DT):
                        for c in range(2):
                            nc.tensor.matmul(
                                pv[:, c * 512:(c + 1) * 512],
                                xT[d][:, s * P:(s + 1) * P],
                                wv_sb[d][:, c * 512:(c + 1) * 512],
                                start=(d == 0), stop=(d == DT - 1))
                    nc.vector.tensor_copy(v_own[s][:], pv[:])
                    nc.scalar.dma_start(
                        v_send[ch].ap()[si * P:(si + 1) * P, :], v_own[s][:])
                nc.gpsimd.collective_compute(
                    "AllGather", mybir.AluOpType.bypass,
                    replica_groups=REPLICA_GROUPS,
                    ins=[v_send[ch].ap().opt()],
                    outs=[v_allc[ch].ap().opt()],
                )

            # ---- q^T projection ----
            qT = []
            for e in range(DT):
                pq = ps1.tile([P, S_OWN], F32, name=f"pq{e}", tag="proj",
                              bufs=3)
                for d in range(DT):
                    for c in range(2):
                        nc.tensor.matmul(
                            pq[:, c * 512:(c + 1) * 512],
                            wq_sb[d][:, e * P:(e + 1) * P],
                            xT[d][:, c * 512:(c + 1) * 512],
                            start=(d == 0), stop=(d == DT - 1))
                qt = sb.tile([P, S_OWN], F32R, name=f"qT{e}", tag="wk0",
                             bufs=8)
                nc.vector.tensor_copy(qt[:], pq[:])
                qT.append(qt)

        # ---- load only the PEER half of the gathers (dynamic row) ----
        kT_peer = []
        epc = DT // KT_CHUNKS
        for ch in range(KT_CHUNKS):
            for ei in range(epc):
                e = ch * epc + ei
                t = sb.tile([P, S_OWN], F32R, name=f"kTp{e}", tag="wv0",
                            bufs=8)
                kT_peer.append(t)
                nc.sync.dma_start(
                    t[:],
                    kt_allc[ch].ap()[bass.ds(peer, 1),
                                     ei * P:(ei + 1) * P, :])
        v_peer = []
        spc = ST // V_CHUNKS
        for ch in range(V_CHUNKS):
            for si in range(spc):
                t = sb.tile([P, D], BF16, name=f"vp{ch}_{si}", tag="wq0",
                            bufs=8)
                v_peer.append(t)
                nc.sync.dma_start(
                    t[:],
                    v_allc[ch].ap()[bass.ds(peer, 1),
                                    si * P:(si + 1) * P, :])

        # ---- attention: flash-style two passes over the key halves.
        # Pass 1 (OWN keys) needs no peer data at all, so it starts
        # right after q-proj (~115us) and fills the window where the
        # baseline stalled waiting for the peer's k^T (~147us). Pass 2
        # (peer keys) starts ~40us after the gather lands -- huge skew
        # margin. Standard flash rescaling makes the result exact:
        #   pass1: m1, l1, O1 = softmax-partial over own keys
        #   pass2: m = max(m1,m2); a = exp((m1-m)/32)
        #          out = (O1*a + O2) / (l1*a + l2)
        kT_half = [kT_own, kT_peer]
        v_half = [v_own, v_peer]
        m1s, l1s, o1s = {}, {}, {}

        with tc.tile_pool(name="ps2", bufs=1, space="PSUM") as ps2:
            state = {}

            def emit_scores(p, sq):
                S_ps = ps2.tile([P, S_OWN], F32, name=f"S{p}_{sq}", tag="S",
                                bufs=2)
                for e in range(DT):
                    for c in range(2):
                        nc.tensor.matmul(
                            S_ps[:, c * 512:(c + 1) * 512],
                            qT[e][:, sq * P:(sq + 1) * P],
                            kT_half[p][e][:, c * 512:(c + 1) * 512],
                            start=(e == 0), stop=(e == DT - 1))
                state[(p, sq)] = S_ps

            def emit_sm1(sq):
                S_ps = state.pop((0, sq))
                m1 = sb.tile([P, 1], F32, name=f"m1_{sq}", tag="m1", bufs=8)
                nc.vector.reduce_max(m1[:], S_ps[:],
                                     axis=mybir.AxisListType.X)
                negm = sb.tile([P, 1], F32, name=f"negm1_{sq}", tag="negm",
                               bufs=2)
                nc.scalar.mul(negm[:], m1[:], -1.0 / 32.0)
                attn = sb.tile([P, S_OWN], BF16, name=f"attn1_{sq}",
                               tag="xa", bufs=3)
                l1 = sb.tile([P, 1], F32, name=f"l1_{sq}", tag="l1", bufs=8)
                nc.scalar.activation(
                    attn[:], S_ps[:], mybir.ActivationFunctionType.Exp,
                    bias=negm[:, 0:1], scale=1.0 / 32.0, accum_out=l1[:])
                m1s[sq], l1s[sq] = m1, l1
                state[(0, sq, "a")] = attn

            def emit_sm2(sq):
                S_ps = state.pop((1, sq))
                m2 = sb.tile([P, 1], F32, name=f"m2_{sq}", tag="m2", bufs=2)
                nc.vector.reduce_max(m2[:], S_ps[:],
                                     axis=mybir.AxisListType.X)
                mm = sb.tile([P, 1], F32, name=f"mm_{sq}", tag="mm", bufs=2)
                nc.vector.tensor_tensor(mm[:], m1s[sq][:], m2[:],
                                        mybir.AluOpType.max)
                negm = sb.tile([P, 1], F32, name=f"negm2_{sq}", tag="negm",
                               bufs=2)
                nc.scalar.mul(negm[:], mm[:], -1.0 / 32.0)
                attn = sb.tile([P, S_OWN], BF16, name=f"attn2_{sq}",
                               tag="xa", bufs=3)
                l2 = sb.tile([P, 1], F32, name=f"l2_{sq}", tag="l2", bufs=2)
                nc.scalar.activation(
                    attn[:], S_ps[:], mybir.ActivationFunctionType.Exp,
                    bias=negm[:, 0:1], scale=1.0 / 32.0, accum_out=l2[:])
                # a = exp((m1 - m)/32); l = l1*a + l2; rl = 1/l
                d1 = sb.tile([P, 1], F32, name=f"d1_{sq}", tag="d1", bufs=2)
                nc.vector.tensor_tensor(d1[:], m1s[sq][:], mm[:],
                                        mybir.AluOpType.subtract)
                alpha = sb.tile([P, 1], F32, name=f"al_{sq}", tag="al",
                                bufs=4)
                nc.scalar.activation(alpha[:], d1[:],
                                     mybir.ActivationFunctionType.Exp,
                                     scale=1.0 / 32.0)
                la = sb.tile([P, 1], F32, name=f"la_{sq}", tag="la", bufs=2)
                nc.vector.tensor_tensor(la[:], l1s[sq][:], alpha[:],
                                        mybir.AluOpType.mult)
                lt = sb.tile([P, 1], F32, name=f"lt_{sq}", tag="lt", bufs=2)
                nc.vector.tensor_tensor(lt[:], la[:], l2[:],
                                        mybir.AluOpType.add)
                rl = sb.tile([P, 1], F32, name=f"rl_{sq}", tag="rl", bufs=4)
                nc.vector.reciprocal(rl[:], lt[:])
                state[(1, sq, "a")] = attn
                state[(sq, "fin")] = (alpha, rl)

            def emit_transp(p, sq):
                attn = state.pop((p, sq, "a"))
                attnT = sb.tile([P, S_OWN], BF16, name=f"aT{p}_{sq}",
                                tag="attnT", bufs=2)
                for t in range(ST):
                    pat = ps2.tile([P, P], BF16, name=f"pat{p}_{sq}_{t}",
                                   tag="pat", bufs=2)
                    nc.tensor.transpose(
                        pat[:], attn[:, t * P:(t + 1) * P], identb[:])
                    nc.vector.tensor_copy(attnT[:, t * P:(t + 1) * P], pat[:])
                state[(p, sq, "T")] = attnT

            def emit_av1(sq):
                attnT = state.pop((0, sq, "T"))
                O_ps = ps2.tile([P, D], F32, name=f"O1_{sq}", tag="O",
                                bufs=1)
                for s in range(ST):
                    for c in range(2):
                        nc.tensor.matmul(
                            O_ps[:, c * 512:(c + 1) * 512],
                            attnT[:, s * P:(s + 1) * P],
                            v_own[s][:, c * 512:(c + 1) * 512],
                            start=(s == 0), stop=(s == ST - 1))
                o1 = sb.tile([P, D], BF16, name=f"o1_{sq}", tag="xT0",
                             bufs=8)
                nc.vector.tensor_copy(o1[:], O_ps[:])
                o1s[sq] = o1

            def emit_av2(sq):
                attnT = state.pop((1, sq, "T"))
                alpha, rl = state.pop((sq, "fin"))
                O_ps = ps2.tile([P, D], F32, name=f"O2_{sq}", tag="O",
                                bufs=1)
                for s in range(ST):
                    for c in range(2):
                        nc.tensor.matmul(
                            O_ps[:, c * 512:(c + 1) * 512],
                            attnT[:, s * P:(s + 1) * P],
                            v_peer[s][:, c * 512:(c + 1) * 512],
                            start=(s == 0), stop=(s == ST - 1))
                o_stage = sb.tile([P, D], F32, name=f"ost{sq}", tag="stage",
                                  bufs=2)
                # out = (o1*alpha + O2) * rl
                nc.vector.scalar_tensor_tensor(
                    o_stage[:], o1s[sq][:], alpha[:, 0:1], O_ps[:],
                    op0=mybir.AluOpType.mult, op1=mybir.AluOpType.add)
                nc.vector.tensor_scalar_mul(o_stage[:], o_stage[:],
                                            rl[:, 0:1])
                nc.scalar.dma_start(out_d.ap()[sq * P:(sq + 1) * P, :],
                                    o_stage[:])

            def emit_pass(p, sm, av):
                for sq in range(ST + 2):
                    if sq < ST:
                        emit_scores(p, sq)
                    if sq >= 2:
                        av(sq - 2)
                    if sq < ST:
                        sm(sq)
                        emit_transp(p, sq)

            emit_pass(0, emit_sm1, emit_av1)
            emit_pass(1, emit_sm2, emit_av2)


_NC_CACHE = {}


def _get_nc():
    if "nc" not in _NC_CACHE:
        _NC_CACHE["nc"] = build_kernel()
    return _NC_CACHE["nc"]


def kernel(x, Wq, Wk, Wv, **_ignored):
    x = np.ascontiguousarray(np.asarray(x, dtype=np.float32))
    Wq = np.ascontiguousarray(np.asarray(Wq, dtype=np.float32))
    Wk = np.ascontiguousarray(np.asarray(Wk, dtype=np.float32))
    Wv = np.ascontiguousarray(np.asarray(Wv, dtype=np.float32))
    nc = _get_nc()
    in_maps = []
    for c in range(NCORES):
        b, h = divmod(c, 2)
        in_maps.append({
            "x": x[b, h * S_OWN:(h + 1) * S_OWN, :],
            "Wq": Wq, "Wk": Wk, "Wv": Wv,
        })
    res = run_bass_kernel_spmd(nc, in_maps, core_ids=list(range(NCORES)))
    out = np.empty((B, S_FULL, D), dtype=np.float32)
    for c in range(NCORES):
        b, h = divmod(c, 2)
        out[b, h * S_OWN:(h + 1) * S_OWN, :] = res.results[c]["out"]
    return out
